# revision 48
# baseline (speedup 1.0000x reference)
"""Trainium2 Bass kernel for nn_CrossAttention (B=8, L=K=512, M=N=P=D=64).

One batch per NeuronCore (8 cores). Host prepares value tensors in bf16
(the kernel's internal precision, same rounding an on-chip cast would do)
and in DMA/compute-friendly layouts:
  vk  -> [K, N, P] bf16  (transposed so the p-contraction is innermost/packed)
  vq  -> [L, M, N] bf16  (natural; n-contraction innermost/packed)
  q,k,vexp -> row-permuted f32 so a [128, T, D] SBUF tile is a contiguous DMA

Per core:
  scoresT[k,l] = scale * (K @ Q^T)            # PE f32, contract D=64
  ET = exp(scoresT)                           # ACT -> bf16
  vkc[k,n] = sum_p vk[k,n,p]*vexp[k,p]        # DVE bf16 mult + tree over p
  tmp[l,n], sums[l] = sum_k ET[k,l]*[vkc|1]   # PE: 16 small matmuls -> PSUM [128,65]
  attn[l,m] = sum_n vq[l,m,n]*tmp[l,n]        # DVE bf16 mult + tree over n
  x = attn/sums + q ; out = LN(x)*gamma+beta  # DVE (+ACT sqrt)

DMA: column-halves of vk/vq split across the two HWDGE queues (sync+scalar);
all bulk triggers issued up-front so both queues stream back-to-back.
"""

import numpy as np

B = 8
L = 512
KK = 512
MM = 64
NN = 64
PP = 64
DD = 64
NCORES = 8

LT = L // 128   # 4 l-tiles
KT = KK // 128  # 4 k-tiles

_CACHE = {}


def _patch_multiwait_split():
    """This environment's walrus accepts only ONE sem-wait per instruction,
    while Tile emits instructions carrying several. Rewrite the BIR JSON just
    before compilation: hoist excess waits onto single-wait NoOps inserted
    immediately before the offending instruction on the same engine."""
    import json

    from concourse import bass_utils, bass2jax

    if getattr(bass_utils, "_multiwait_split_patched", False):
        return

    orig = bass_utils.compile_bir_kernel

    def _split(bir_json):
        if isinstance(bir_json, bytes):
            m = json.loads(bir_json.decode())
        else:
            m = json.loads(bir_json)
        cnt = 0
        for fn in m["functions"]:
            for bb in fn["blocks"]:
                insts = bb["instructions"]
                out = []
                for inst in insts:
                    si = inst.get("sync_info")
                    waits = si.get("on_wait", []) if si else []
                    if len(waits) > 1:
                        for w in waits[:-1]:
                            cnt += 1
                            out.append(
                                {
                                    "name": f"WS-{cnt}-{inst['name']}",
                                    "opcode": "NoOp",
                                    "engine": inst["engine"],
                                    "ins": [],
                                    "outs": [],
                                    "debug": inst.get("debug", 0),
                                    "sync_info": {
                                        "on_update": [],
                                        "on_wait": [w],
                                    },
                                }
                            )
                        si["on_wait"] = [waits[-1]]
                    out.append(inst)
                bb["instructions"] = out
        return json.dumps(m).encode()

    def patched(bir_json, tmpdir, neff_name="file.neff", **kw):
        return orig(_split(bir_json), tmpdir, neff_name=neff_name, **kw)

    bass_utils.compile_bir_kernel = patched
    bass2jax.compile_bir_kernel = patched
    bass_utils._multiwait_split_patched = True


def _build_nc():
    import contextlib

    import concourse.bass as bass
    import concourse.tile as tile
    from concourse import mybir
    from concourse.masks import make_identity

    _patch_multiwait_split()

    f32 = mybir.dt.float32
    bf16 = mybir.dt.bfloat16
    Alu = mybir.AluOpType
    Act = mybir.ActivationFunctionType

    nc = bass.Bass()
    # q/k/vexp row-permuted on host: row index = p*T + t  (partition, tile)
    q_d = nc.dram_tensor("q", [128, LT * DD], f32, kind="ExternalInput")
    k_d = nc.dram_tensor("k", [128, KT * DD], f32, kind="ExternalInput")
    vexp_d = nc.dram_tensor("vexp", [128, KT * PP], f32, kind="ExternalInput")
    vq_d = nc.dram_tensor("vq", [L, MM * NN], bf16, kind="ExternalInput")
    vk_d = nc.dram_tensor("vk", [KK, NN * PP], bf16, kind="ExternalInput")
    scale_d = nc.dram_tensor("scale", [1, 1], f32, kind="ExternalInput")
    gamma_d = nc.dram_tensor("ln_gamma", [1, DD], f32, kind="ExternalInput")
    beta_d = nc.dram_tensor("ln_beta", [1, DD], f32, kind="ExternalInput")
    # output row-permuted the same way; host un-permutes
    out_d = nc.dram_tensor("out", [128, LT * MM], f32, kind="ExternalOutput")

    HALFC = NN * PP // 2  # column-half of a value tile

    with tile.TileContext(nc) as tc:
        lp_cm = nc.allow_low_precision("bf16 value-path partial sums")
        with lp_cm, contextlib.ExitStack() as ctx:
            const = ctx.enter_context(tc.tile_pool(name="const", bufs=1))
            vkb_pool = ctx.enter_context(tc.tile_pool(name="vkb", bufs=1))
            vqb_pool = ctx.enter_context(tc.tile_pool(name="vqb", bufs=1))
            prod_pool = ctx.enter_context(tc.tile_pool(name="prod", bufs=1))
            small = ctx.enter_context(tc.tile_pool(name="small", bufs=2))
            ps_scores = ctx.enter_context(
                tc.tile_pool(name="ps_s", bufs=2, space="PSUM")
            )
            ps_tmp = ctx.enter_context(tc.tile_pool(name="ps_tmp", bufs=1, space="PSUM"))
            ps_tr = ctx.enter_context(tc.tile_pool(name="ps_tr", bufs=2, space="PSUM"))

            # ---- DMA. Order matters: the engine issuing a queue's triggers
            # stalls at a trigger whose 4-deep ring slot isn't free, blocking
            # everything behind it in that engine's stream. So: vexp + vk
            # first (they gate the DVE critical path), q/k after vk (the
            # ET chain has slack), vq-h1 triggers interleaved into the exp
            # loop so they never block scalar compute. ----
            # pair tiles: two k-tiles (or l-tiles) adjacent in SBUF so one 4D
            # DVE instruction processes both (halves instruction overhead)
            vk_pairs = []
            for p in range(KT // 2):
                t = vkb_pool.tile([128, 2, NN, PP], bf16, tag=f"vkp{p}")
                vk_pairs.append(t)
            vq_pairs = []
            for p in range(LT // 2):
                t = vqb_pool.tile([128, 2, MM, NN], bf16, tag=f"vqp{p}")
                vq_pairs.append(t)

            def vk_tile(i):  # [128, NN, PP] view of tile i
                return vk_pairs[i // 2][:, i % 2]

            def vq_tile(j):
                return vq_pairs[j // 2][:, j % 2]

            vk_tiles = [vk_tile(i) for i in range(KT)]
            vq_tiles = [vq_tile(j) for j in range(LT)]

            # sync queue: vexp, vk h0 x4, q, k, vq h0 x4 (stores at the end)
            # vk0 quartered: each n-half comes one piece per queue, so the
            # first n-half (cols 0:2048) lands as soon as both queues finish
            # their FIRST transfer -> the first (half-)mult starts ~4us sooner
            # vk0 h0 first (it gates the first DVE mult), vexp second
            nc.sync.dma_start(
                out=vk_tiles[0][:].rearrange("a n p -> a (n p)")[:, 0:HALFC],
                in_=vk_d[0:128, 0:HALFC],
            )
            vexp_nat = const.tile([128, KT, PP], f32)
            nc.sync.dma_start(out=vexp_nat, in_=vexp_d[:])
            # drain: let vk0-h0 own the queue's DMA engines until it lands
            # (descriptor processing round-robins across outstanding ring
            # entries, so piling on more transfers delays the first one)
            nc.sync.drain()
            for i in range(1, KT):
                nc.sync.dma_start(
                    out=vk_tiles[i][:].rearrange("a n p -> a (n p)")[:, 0:HALFC],
                    in_=vk_d[i * 128 : (i + 1) * 128, 0:HALFC],
                )
            q_nat = const.tile([128, LT, DD], f32)
            nc.sync.dma_start(out=q_nat, in_=q_d[:])
            k_nat = const.tile([128, KT, DD], f32)
            nc.sync.dma_start(out=k_nat, in_=k_d[:])
            for j in range(LT):
                nc.sync.dma_start(
                    out=vq_tiles[j][:].rearrange("a m n -> a (m n)")[:, 0:HALFC],
                    in_=vq_d[j * 128 : (j + 1) * 128, 0:HALFC],
                )

            # scalar queue: vk h1 x2 first, vexp cast, consts, vk h1 x2
            nc.scalar.dma_start(
                out=vk_tiles[0][:].rearrange("a n p -> a (n p)")[:, HALFC:],
                in_=vk_d[0:128, HALFC:],
            )
            nc.scalar.drain()
            nc.scalar.dma_start(
                out=vk_tiles[1][:].rearrange("a n p -> a (n p)")[:, HALFC:],
                in_=vk_d[128:256, HALFC:],
            )
            # vexp cast ASAP: it gates the first DVE mult; must not sit
            # behind ring-slot-blocked DMA triggers in the scalar stream
            vexp_b = const.tile([128, KT, PP], bf16)
            nc.scalar.copy(vexp_b, vexp_nat)
            scale_bc = const.tile([128, 1], f32)
            nc.scalar.dma_start(out=scale_bc, in_=scale_d[:].to_broadcast([128, 1]))
            gamma_bc = const.tile([128, DD], f32)
            nc.scalar.dma_start(out=gamma_bc, in_=gamma_d[:].to_broadcast([128, DD]))
            beta_bc = const.tile([128, DD], f32)
            nc.scalar.dma_start(out=beta_bc, in_=beta_d[:].to_broadcast([128, DD]))
            for i in range(2, KT):
                nc.scalar.dma_start(
                    out=vk_tiles[i][:].rearrange("a n p -> a (n p)")[:, HALFC:],
                    in_=vk_d[i * 128 : (i + 1) * 128, HALFC:],
                )


            # ---- constants ----
            identity = const.tile([128, 128], f32)
            make_identity(nc, identity)
            zero_t = const.tile([128, 1], f32)
            nc.vector.memset(zero_t, 0.0)
            eps_t = const.tile([128, 1], f32)
            nc.vector.memset(eps_t, 1e-3)

            # ---- qT, kT via PE transpose ----
            qT = const.tile([64, L], f32)
            kT = const.tile([64, KK], f32)
            for i in range(LT):
                pq = ps_tr.tile([64, 128], f32, tag="tr")
                nc.tensor.transpose(pq, q_nat[:, i, :], identity)
                nc.scalar.copy(qT[:, i * 128 : (i + 1) * 128], pq)
            for i in range(KT):
                pk = ps_tr.tile([64, 128], f32, tag="tr")
                nc.tensor.transpose(pk, k_nat[:, i, :], identity)
                nc.scalar.copy(kT[:, i * 128 : (i + 1) * 128], pk)

            # ---- scores -> ET (bf16, [k, l] layout); vq h1 triggers ride
            # between exps so the scalar queue keeps streaming ----
            ET = const.tile([128, KT, L], bf16)
            for i in range(KT):
                ps_s = ps_scores.tile([128, L], f32, tag="sc")
                nc.tensor.matmul(
                    ps_s, lhsT=kT[:, i * 128 : (i + 1) * 128], rhs=qT[:],
                    start=True, stop=True,
                )
                nc.scalar.activation(
                    ET[:, i, :], ps_s, func=Act.Exp, bias=zero_t[:], scale=scale_bc[:],
                )
                nc.scalar.dma_start(
                    out=vq_tiles[i][:].rearrange("a m n -> a (m n)")[:, HALFC:],
                    in_=vq_d[i * 128 : (i + 1) * 128, HALFC:],
                )

            # ---- vkc[k, n] (+ ones col 64): bf16 mult + tree over p (inner),
            # two k-tiles per instruction via 4D APs ----
            vkc = const.tile([128, KT, NN + 1], bf16)
            nc.vector.memset(vkc, 1.0)  # col 64 stays ones
            def vkc_block(i, n0, nn):
                # products + tree over p for n-range [n0, n0+nn) of k-tile i
                pr = prod_pool.tile([128, nn, PP], bf16, tag=f"prod{nn}")
                nc.vector.tensor_tensor(
                    pr[:],
                    vk_tile(i)[:, n0 : n0 + nn, :],
                    vexp_b[:, i, None, :].to_broadcast([128, nn, PP]),
                    Alu.mult,
                )
                cur = pr
                w = PP // 2
                while w >= 1:
                    if w == 1:
                        nxt = vkc[:, i, n0 : n0 + nn, None]
                    else:
                        nxt = prod_pool.tile([128, nn, w], bf16, tag=f"atree{nn}_{w}")
                    nc.vector.tensor_tensor(
                        nxt[:], cur[:, :, 0:w], cur[:, :, w : 2 * w], Alu.add
                    )
                    cur = nxt
                    w //= 2

            for i in range(KT):
                vkc_block(i, 0, NN)

            # ---- tmp[l, n] + sums[l]: 16 small matmuls, accumulate over k ----
            tmp_ps = []
            for j in range(LT):
                tmp_ps_j = ps_tmp.tile([128, NN + 1], f32, tag=f"tmp{j}")
                tmp_ps.append(tmp_ps_j)
            for i in range(KT):
                for j in range(LT):
                    nc.tensor.matmul(
                        tmp_ps[j],
                        lhsT=ET[:, i, j * 128 : (j + 1) * 128],
                        rhs=vkc[:, i, :],
                        start=(i == 0), stop=(i == KT - 1),
                    )

            # ---- per l-tile: tmp -> SBUF bf16, recip sums, phase C, LN.
            # The LN tail of tile j is deferred until after tile j+1's
            # mult/tree so the ACT sqrt roundtrip hides under DVE work. ----
            recip_col = const.tile([128, LT], f32)

            def ln_tail2(pe0, pe1):
                # paired LN tail: per-j normalize, then one gamma/beta pass
                # and one store for both l-tiles
                p = pe0[3] // 2
                xn2 = small.tile([128, 2, MM], f32, tag="xn2")
                for jj, (x, mv, sd, j) in enumerate((pe0, pe1)):
                    rstd = small.tile([128, 1], f32, tag=f"rstd{jj}")
                    nc.vector.reciprocal(rstd, sd)
                    nc.vector.tensor_scalar(
                        out=xn2[:, jj, :], in0=x, scalar1=mv[:, 0:1], scalar2=rstd,
                        op0=Alu.subtract, op1=Alu.mult,
                    )
                xg2 = small.tile([128, 2, MM], f32, tag="xg2")
                nc.vector.tensor_tensor(
                    xg2[:], xn2[:],
                    gamma_bc[:, None, :].to_broadcast([128, 2, MM]), Alu.mult,
                )
                out2 = small.tile([128, 2, MM], f32, tag="out2")
                nc.vector.tensor_tensor(
                    out2[:], xg2[:],
                    beta_bc[:, None, :].to_broadcast([128, 2, MM]), Alu.add,
                )
                nc.sync.dma_start(
                    out=out_d[:, 2 * p * MM : (2 * p + 2) * MM], in_=out2
                )

            pends = []
            for p in range(LT // 2):
                tmp_sbp = small.tile([128, 2, NN], bf16, tag=f"tmp_sbp{p}")
                for jj in range(2):
                    j = 2 * p + jj
                    nc.scalar.copy(tmp_sbp[:, jj, :], tmp_ps[j][:, 0:NN])
                    nc.vector.reciprocal(
                        recip_col[:, j : j + 1], tmp_ps[j][:, NN : NN + 1]
                    )

                pr2 = prod_pool.tile([128, 2, MM, NN], bf16, tag="prod2")
                nc.vector.tensor_tensor(
                    pr2[:],
                    vq_pairs[p][:],
                    tmp_sbp[:, :, None, :].to_broadcast([128, 2, MM, NN]),
                    Alu.mult,
                )
                attn2 = small.tile([128, 2, MM], f32, tag="attn2")
                cur = pr2
                w = NN // 2
                while w >= 1:
                    if w == 1:
                        nxt = attn2[:, :, :, None]
                    else:
                        nxt = prod_pool.tile([128, 2, MM, w], bf16, tag=f"ctree{w}")
                    nc.vector.tensor_tensor(
                        nxt[:], cur[:, :, :, 0:w], cur[:, :, :, w : 2 * w], Alu.add
                    )
                    cur = nxt
                    w //= 2

                for jj in range(2):
                    j = 2 * p + jj
                    # x = attn * (1/sums) + q
                    x = small.tile([128, MM], f32, tag=f"x{j}")
                    nc.vector.scalar_tensor_tensor(
                        out=x, in0=attn2[:, jj, :], scalar=recip_col[:, j : j + 1],
                        in1=q_nat[:, j, :], op0=Alu.mult, op1=Alu.add,
                    )
                    stats = small.tile([128, 6], f32, tag="stats")
                    nc.vector.bn_stats(out=stats, in_=x[:])
                    mv = small.tile([128, 2], f32, tag=f"mv{j}")
                    nc.vector.bn_aggr(out=mv, in_=stats[:])
                    sd = small.tile([128, 1], f32, tag=f"sd{j}")
                    nc.scalar.activation(
                        sd, mv[:, 1:2], func=Act.Sqrt, bias=eps_t[:], scale=1.0
                    )
                    pends.append((x, mv, sd, j))
                if p > 0:
                    ln_tail2(pends[0], pends[1])
                    pends = pends[2:]
            ln_tail2(pends[0], pends[1])

    return nc


def _get_nc():
    if "nc" not in _CACHE:
        _CACHE["nc"] = _build_nc()
    return _CACHE["nc"]


LAST_EXEC_NS = None
LAST_PROFILE_JSON = None


def kernel(q, k, vq, vk, vexp, scale, ln_gamma, ln_beta):
    import os

    import ml_dtypes
    from concourse import bass_utils

    nc = _get_nc()
    bf16 = ml_dtypes.bfloat16

    q = np.asarray(q, dtype=np.float32)
    k = np.asarray(k, dtype=np.float32)
    vq = np.asarray(vq, dtype=np.float32)
    vk = np.asarray(vk, dtype=np.float32)
    vexp = np.asarray(vexp, dtype=np.float32)

    # row-permute q/k/vexp: device row p holds original rows [p, 128+p, 256+p, 384+p]
    def perm_rows(x, t):  # [B, T*128, D] -> [B, 128, T*D]
        return np.ascontiguousarray(
            x.reshape(B, t, 128, -1).transpose(0, 2, 1, 3).reshape(B, 128, -1)
        )

    q_p = perm_rows(q, LT)
    k_p = perm_rows(k, KT)
    vexp_p = perm_rows(vexp, KT)
    # value tensors to bf16 (kernel-internal precision), vk transposed to [K, N, P]
    vk_b = np.ascontiguousarray(vk.transpose(0, 1, 3, 2)).astype(bf16).reshape(
        B, KK, NN * PP
    )
    vq_b = vq.astype(bf16).reshape(B, L, MM * NN)

    scale_arr = np.asarray(scale, dtype=np.float32).reshape(1, 1)
    gamma_arr = np.asarray(ln_gamma, dtype=np.float32).reshape(1, DD)
    beta_arr = np.asarray(ln_beta, dtype=np.float32).reshape(1, DD)

    in_maps = [
        {
            "q": q_p[c],
            "k": k_p[c],
            "vq": vq_b[c],
            "vk": vk_b[c],
            "vexp": vexp_p[c],
            "scale": scale_arr,
            "ln_gamma": gamma_arr,
            "ln_beta": beta_arr,
        }
        for c in range(NCORES)
    ]
    trace = bool(os.environ.get("KERNEL_TRACE"))
    kw = {}
    if trace:
        kw = dict(trace=True, tmpdir=os.environ.get("KERNEL_TRACE_DIR") or None)
    res = bass_utils.run_bass_kernel_spmd(
        nc, in_maps, core_ids=list(range(NCORES)), **kw
    )
    if trace:
        global LAST_EXEC_NS, LAST_PROFILE_JSON
        LAST_EXEC_NS = res.exec_time_ns
        LAST_PROFILE_JSON = res.profile_json
    # un-permute output rows: [128, LT*MM] -> [L, MM]
    out_p = np.stack([res.results[c]["out"] for c in range(NCORES)], axis=0)
    out = (
        out_p.reshape(B, 128, LT, MM).transpose(0, 2, 1, 3).reshape(B, L, MM)
    )
    return np.ascontiguousarray(out.astype(np.float32))


# revision 49
# speedup vs baseline: 1.1651x; 1.1651x over previous
"""Trainium2 Bass kernel for nn_CrossAttention (B=8, L=K=512, M=N=P=D=64).

One batch per NeuronCore (8 cores). Host prepares value tensors in bf16
(the kernel's internal precision, same rounding an on-chip cast would do)
and in DMA/compute-friendly layouts:
  vk  -> [K, N, P] bf16  (transposed so the p-contraction is innermost/packed)
  vq  -> [L, M, N] bf16  (natural; n-contraction innermost/packed)
  q,k,vexp -> row-permuted f32 so a [128, T, D] SBUF tile is a contiguous DMA

Per core:
  scoresT[k,l] = scale * (K @ Q^T)            # PE f32, contract D=64
  ET = exp(scoresT)                           # ACT -> bf16
  vkc[k,n] = sum_p vk[k,n,p]*vexp[k,p]        # DVE bf16 mult + tree over p
  tmp[l,n], sums[l] = sum_k ET[k,l]*[vkc|1]   # PE: 16 small matmuls -> PSUM [128,65]
  attn[l,m] = sum_n vq[l,m,n]*tmp[l,n]        # DVE bf16 mult + tree over n
  x = attn/sums + q ; out = LN(x)*gamma+beta  # DVE (+ACT sqrt)

DMA: column-halves of vk/vq split across the two HWDGE queues (sync+scalar);
all bulk triggers issued up-front so both queues stream back-to-back.
"""

import numpy as np

B = 8
L = 512
KK = 512
MM = 64
NN = 64
PP = 64
DD = 64
NCORES = 8

LT = L // 128   # 4 l-tiles
KT = KK // 128  # 4 k-tiles

_CACHE = {}


def _patch_multiwait_split():
    """This environment's walrus accepts only ONE sem-wait per instruction,
    while Tile emits instructions carrying several. Rewrite the BIR JSON just
    before compilation: hoist excess waits onto single-wait NoOps inserted
    immediately before the offending instruction on the same engine."""
    import json

    from concourse import bass_utils, bass2jax

    if getattr(bass_utils, "_multiwait_split_patched", False):
        return

    orig = bass_utils.compile_bir_kernel

    def _split(bir_json):
        if isinstance(bir_json, bytes):
            m = json.loads(bir_json.decode())
        else:
            m = json.loads(bir_json)
        cnt = 0
        for fn in m["functions"]:
            for bb in fn["blocks"]:
                insts = bb["instructions"]
                out = []
                for inst in insts:
                    si = inst.get("sync_info")
                    waits = si.get("on_wait", []) if si else []
                    if len(waits) > 1:
                        for w in waits[:-1]:
                            cnt += 1
                            out.append(
                                {
                                    "name": f"WS-{cnt}-{inst['name']}",
                                    "opcode": "NoOp",
                                    "engine": inst["engine"],
                                    "ins": [],
                                    "outs": [],
                                    "debug": inst.get("debug", 0),
                                    "sync_info": {
                                        "on_update": [],
                                        "on_wait": [w],
                                    },
                                }
                            )
                        si["on_wait"] = [waits[-1]]
                    out.append(inst)
                bb["instructions"] = out
        return json.dumps(m).encode()

    def patched(bir_json, tmpdir, neff_name="file.neff", **kw):
        return orig(_split(bir_json), tmpdir, neff_name=neff_name, **kw)

    bass_utils.compile_bir_kernel = patched
    bass2jax.compile_bir_kernel = patched
    bass_utils._multiwait_split_patched = True


def _build_nc():
    import contextlib

    import concourse.bass as bass
    import concourse.tile as tile
    from concourse import mybir
    from concourse.masks import make_identity

    _patch_multiwait_split()

    f32 = mybir.dt.float32
    bf16 = mybir.dt.bfloat16
    Alu = mybir.AluOpType
    Act = mybir.ActivationFunctionType

    nc = bass.Bass()
    # q/k/vexp row-permuted on host: row index = p*T + t  (partition, tile)
    q_d = nc.dram_tensor("q", [128, LT * DD], f32, kind="ExternalInput")
    k_d = nc.dram_tensor("k", [128, KT * DD], f32, kind="ExternalInput")
    vexp_d = nc.dram_tensor("vexp", [128, KT * PP], f32, kind="ExternalInput")
    vq_d = nc.dram_tensor("vq", [L, MM * NN], bf16, kind="ExternalInput")
    vk_d = nc.dram_tensor("vk", [KK, NN * PP], bf16, kind="ExternalInput")
    scale_d = nc.dram_tensor("scale", [1, 1], f32, kind="ExternalInput")
    gamma_d = nc.dram_tensor("ln_gamma", [1, DD], f32, kind="ExternalInput")
    beta_d = nc.dram_tensor("ln_beta", [1, DD], f32, kind="ExternalInput")
    # output row-permuted the same way; host un-permutes
    out_d = nc.dram_tensor("out", [128, LT * MM], f32, kind="ExternalOutput")

    HALFC = NN * PP // 2  # column-half of a value tile

    with tile.TileContext(nc) as tc:
        lp_cm = nc.allow_low_precision("bf16 value-path partial sums")
        with lp_cm, contextlib.ExitStack() as ctx:
            const = ctx.enter_context(tc.tile_pool(name="const", bufs=1))
            vkb_pool = ctx.enter_context(tc.tile_pool(name="vkb", bufs=1))
            vqb_pool = ctx.enter_context(tc.tile_pool(name="vqb", bufs=1))
            prod_pool = ctx.enter_context(tc.tile_pool(name="prod", bufs=1))
            small = ctx.enter_context(tc.tile_pool(name="small", bufs=2))
            ps_scores = ctx.enter_context(
                tc.tile_pool(name="ps_s", bufs=2, space="PSUM")
            )
            ps_tmp = ctx.enter_context(tc.tile_pool(name="ps_tmp", bufs=1, space="PSUM"))
            ps_tr = ctx.enter_context(tc.tile_pool(name="ps_tr", bufs=2, space="PSUM"))

            # ---- DMA. Order matters: the engine issuing a queue's triggers
            # stalls at a trigger whose 4-deep ring slot isn't free, blocking
            # everything behind it in that engine's stream. So: vexp + vk
            # first (they gate the DVE critical path), q/k after vk (the
            # ET chain has slack), vq-h1 triggers interleaved into the exp
            # loop so they never block scalar compute. ----
            # pair tiles: two k-tiles (or l-tiles) adjacent in SBUF so one 4D
            # DVE instruction processes both (halves instruction overhead)
            vk_pairs = []
            for p in range(KT // 2):
                t = vkb_pool.tile([128, 2, NN, PP], bf16, tag=f"vkp{p}")
                vk_pairs.append(t)
            vq_pairs = []
            for p in range(LT // 2):
                t = vqb_pool.tile([128, 2, MM, NN], bf16, tag=f"vqp{p}")
                vq_pairs.append(t)

            def vk_tile(i):  # [128, NN, PP] view of tile i
                return vk_pairs[i // 2][:, i % 2]

            def vq_tile(j):
                return vq_pairs[j // 2][:, j % 2]

            vk_tiles = [vk_tile(i) for i in range(KT)]
            vq_tiles = [vq_tile(j) for j in range(LT)]

            # sync queue: vexp, vk h0 x4, q, k, vq h0 x4 (stores at the end)
            # vk0 quartered: each n-half comes one piece per queue, so the
            # first n-half (cols 0:2048) lands as soon as both queues finish
            # their FIRST transfer -> the first (half-)mult starts ~4us sooner
            # vk0 h0 first (it gates the first DVE mult), vexp second
            nc.sync.dma_start(
                out=vk_tiles[0][:].rearrange("a n p -> a (n p)")[:, 0:HALFC],
                in_=vk_d[0:128, 0:HALFC],
            )
            vexp_nat = const.tile([128, KT, PP], f32)
            nc.sync.dma_start(out=vexp_nat, in_=vexp_d[:])
            for i in range(1, KT):
                nc.sync.dma_start(
                    out=vk_tiles[i][:].rearrange("a n p -> a (n p)")[:, 0:HALFC],
                    in_=vk_d[i * 128 : (i + 1) * 128, 0:HALFC],
                )
            q_nat = const.tile([128, LT, DD], f32)
            nc.sync.dma_start(out=q_nat, in_=q_d[:])
            k_nat = const.tile([128, KT, DD], f32)
            nc.sync.dma_start(out=k_nat, in_=k_d[:])
            for j in range(LT):
                nc.sync.dma_start(
                    out=vq_tiles[j][:].rearrange("a m n -> a (m n)")[:, 0:HALFC],
                    in_=vq_d[j * 128 : (j + 1) * 128, 0:HALFC],
                )

            # scalar queue: vk h1 x2 first, vexp cast, consts, vk h1 x2
            nc.scalar.dma_start(
                out=vk_tiles[0][:].rearrange("a n p -> a (n p)")[:, HALFC:],
                in_=vk_d[0:128, HALFC:],
            )
            nc.scalar.dma_start(
                out=vk_tiles[1][:].rearrange("a n p -> a (n p)")[:, HALFC:],
                in_=vk_d[128:256, HALFC:],
            )
            # vexp cast ASAP: it gates the first DVE mult; must not sit
            # behind ring-slot-blocked DMA triggers in the scalar stream
            vexp_b = const.tile([128, KT, PP], bf16)
            nc.scalar.copy(vexp_b, vexp_nat)
            scale_bc = const.tile([128, 1], f32)
            nc.scalar.dma_start(out=scale_bc, in_=scale_d[:].to_broadcast([128, 1]))
            gamma_bc = const.tile([128, DD], f32)
            nc.scalar.dma_start(out=gamma_bc, in_=gamma_d[:].to_broadcast([128, DD]))
            beta_bc = const.tile([128, DD], f32)
            nc.scalar.dma_start(out=beta_bc, in_=beta_d[:].to_broadcast([128, DD]))
            for i in range(2, KT):
                nc.scalar.dma_start(
                    out=vk_tiles[i][:].rearrange("a n p -> a (n p)")[:, HALFC:],
                    in_=vk_d[i * 128 : (i + 1) * 128, HALFC:],
                )


            # ---- constants ----
            identity = const.tile([128, 128], f32)
            make_identity(nc, identity)
            zero_t = const.tile([128, 1], f32)
            nc.vector.memset(zero_t, 0.0)
            eps_t = const.tile([128, 1], f32)
            nc.vector.memset(eps_t, 1e-3)

            # ---- qT, kT via PE transpose ----
            qT = const.tile([64, L], f32)
            kT = const.tile([64, KK], f32)
            for i in range(LT):
                pq = ps_tr.tile([64, 128], f32, tag="tr")
                nc.tensor.transpose(pq, q_nat[:, i, :], identity)
                nc.scalar.copy(qT[:, i * 128 : (i + 1) * 128], pq)
            for i in range(KT):
                pk = ps_tr.tile([64, 128], f32, tag="tr")
                nc.tensor.transpose(pk, k_nat[:, i, :], identity)
                nc.scalar.copy(kT[:, i * 128 : (i + 1) * 128], pk)

            # ---- scores -> ET (bf16, [k, l] layout); vq h1 triggers ride
            # between exps so the scalar queue keeps streaming ----
            ET = const.tile([128, KT, L], bf16)
            for i in range(KT):
                ps_s = ps_scores.tile([128, L], f32, tag="sc")
                nc.tensor.matmul(
                    ps_s, lhsT=kT[:, i * 128 : (i + 1) * 128], rhs=qT[:],
                    start=True, stop=True,
                )
                nc.scalar.activation(
                    ET[:, i, :], ps_s, func=Act.Exp, bias=zero_t[:], scale=scale_bc[:],
                )
                nc.scalar.dma_start(
                    out=vq_tiles[i][:].rearrange("a m n -> a (m n)")[:, HALFC:],
                    in_=vq_d[i * 128 : (i + 1) * 128, HALFC:],
                )

            # ---- vkc[k, n] (+ ones col 64): bf16 mult + tree over p (inner),
            # two k-tiles per instruction via 4D APs ----
            vkc = const.tile([128, KT, NN + 1], bf16)
            nc.vector.memset(vkc, 1.0)  # col 64 stays ones
            def vkc_block(i, n0, nn):
                # products + tree over p for n-range [n0, n0+nn) of k-tile i
                pr = prod_pool.tile([128, nn, PP], bf16, tag=f"prod{nn}")
                nc.vector.tensor_tensor(
                    pr[:],
                    vk_tile(i)[:, n0 : n0 + nn, :],
                    vexp_b[:, i, None, :].to_broadcast([128, nn, PP]),
                    Alu.mult,
                )
                cur = pr
                w = PP // 2
                while w >= 1:
                    if w == 1:
                        nxt = vkc[:, i, n0 : n0 + nn, None]
                    else:
                        nxt = prod_pool.tile([128, nn, w], bf16, tag=f"atree{nn}_{w}")
                    nc.vector.tensor_tensor(
                        nxt[:], cur[:, :, 0:w], cur[:, :, w : 2 * w], Alu.add
                    )
                    cur = nxt
                    w //= 2

            for i in range(KT):
                vkc_block(i, 0, NN)

            # ---- tmp[l, n] + sums[l]: 16 small matmuls, accumulate over k ----
            tmp_ps = []
            for j in range(LT):
                tmp_ps_j = ps_tmp.tile([128, NN + 1], f32, tag=f"tmp{j}")
                tmp_ps.append(tmp_ps_j)
            for i in range(KT):
                for j in range(LT):
                    nc.tensor.matmul(
                        tmp_ps[j],
                        lhsT=ET[:, i, j * 128 : (j + 1) * 128],
                        rhs=vkc[:, i, :],
                        start=(i == 0), stop=(i == KT - 1),
                    )

            # ---- per l-tile: tmp -> SBUF bf16, recip sums, phase C, LN.
            # The LN tail of tile j is deferred until after tile j+1's
            # mult/tree so the ACT sqrt roundtrip hides under DVE work. ----
            recip_col = const.tile([128, LT], f32)

            def ln_tail2(pe0, pe1):
                # paired LN tail: per-j normalize, then one gamma/beta pass
                # and one store for both l-tiles
                p = pe0[3] // 2
                xn2 = small.tile([128, 2, MM], f32, tag="xn2")
                for jj, (x, mv, sd, j) in enumerate((pe0, pe1)):
                    rstd = small.tile([128, 1], f32, tag=f"rstd{jj}")
                    nc.vector.reciprocal(rstd, sd)
                    nc.vector.tensor_scalar(
                        out=xn2[:, jj, :], in0=x, scalar1=mv[:, 0:1], scalar2=rstd,
                        op0=Alu.subtract, op1=Alu.mult,
                    )
                xg2 = small.tile([128, 2, MM], f32, tag="xg2")
                nc.vector.tensor_tensor(
                    xg2[:], xn2[:],
                    gamma_bc[:, None, :].to_broadcast([128, 2, MM]), Alu.mult,
                )
                out2 = small.tile([128, 2, MM], f32, tag="out2")
                nc.vector.tensor_tensor(
                    out2[:], xg2[:],
                    beta_bc[:, None, :].to_broadcast([128, 2, MM]), Alu.add,
                )
                nc.sync.dma_start(
                    out=out_d[:, 2 * p * MM : (2 * p + 2) * MM], in_=out2
                )

            pends = []
            for p in range(LT // 2):
                tmp_sbp = small.tile([128, 2, NN], bf16, tag=f"tmp_sbp{p}")
                for jj in range(2):
                    j = 2 * p + jj
                    nc.scalar.copy(tmp_sbp[:, jj, :], tmp_ps[j][:, 0:NN])
                    nc.vector.reciprocal(
                        recip_col[:, j : j + 1], tmp_ps[j][:, NN : NN + 1]
                    )

                pr2 = prod_pool.tile([128, 2, MM, NN], bf16, tag="prod2")
                nc.vector.tensor_tensor(
                    pr2[:],
                    vq_pairs[p][:],
                    tmp_sbp[:, :, None, :].to_broadcast([128, 2, MM, NN]),
                    Alu.mult,
                )
                attn2 = small.tile([128, 2, MM], f32, tag="attn2")
                cur = pr2
                w = NN // 2
                while w >= 1:
                    if w == 1:
                        nxt = attn2[:, :, :, None]
                    else:
                        nxt = prod_pool.tile([128, 2, MM, w], bf16, tag=f"ctree{w}")
                    nc.vector.tensor_tensor(
                        nxt[:], cur[:, :, :, 0:w], cur[:, :, :, w : 2 * w], Alu.add
                    )
                    cur = nxt
                    w //= 2

                for jj in range(2):
                    j = 2 * p + jj
                    # x = attn * (1/sums) + q
                    x = small.tile([128, MM], f32, tag=f"x{j}")
                    nc.vector.scalar_tensor_tensor(
                        out=x, in0=attn2[:, jj, :], scalar=recip_col[:, j : j + 1],
                        in1=q_nat[:, j, :], op0=Alu.mult, op1=Alu.add,
                    )
                    stats = small.tile([128, 6], f32, tag="stats")
                    nc.vector.bn_stats(out=stats, in_=x[:])
                    mv = small.tile([128, 2], f32, tag=f"mv{j}")
                    nc.vector.bn_aggr(out=mv, in_=stats[:])
                    sd = small.tile([128, 1], f32, tag=f"sd{j}")
                    nc.scalar.activation(
                        sd, mv[:, 1:2], func=Act.Sqrt, bias=eps_t[:], scale=1.0
                    )
                    pends.append((x, mv, sd, j))
                if p > 0:
                    ln_tail2(pends[0], pends[1])
                    pends = pends[2:]
            ln_tail2(pends[0], pends[1])

    return nc


def _get_nc():
    if "nc" not in _CACHE:
        _CACHE["nc"] = _build_nc()
    return _CACHE["nc"]


LAST_EXEC_NS = None
LAST_PROFILE_JSON = None


def kernel(q, k, vq, vk, vexp, scale, ln_gamma, ln_beta):
    import os

    import ml_dtypes
    from concourse import bass_utils

    nc = _get_nc()
    bf16 = ml_dtypes.bfloat16

    q = np.asarray(q, dtype=np.float32)
    k = np.asarray(k, dtype=np.float32)
    vq = np.asarray(vq, dtype=np.float32)
    vk = np.asarray(vk, dtype=np.float32)
    vexp = np.asarray(vexp, dtype=np.float32)

    # row-permute q/k/vexp: device row p holds original rows [p, 128+p, 256+p, 384+p]
    def perm_rows(x, t):  # [B, T*128, D] -> [B, 128, T*D]
        return np.ascontiguousarray(
            x.reshape(B, t, 128, -1).transpose(0, 2, 1, 3).reshape(B, 128, -1)
        )

    q_p = perm_rows(q, LT)
    k_p = perm_rows(k, KT)
    vexp_p = perm_rows(vexp, KT)
    # value tensors to bf16 (kernel-internal precision), vk transposed to [K, N, P]
    vk_b = np.ascontiguousarray(vk.transpose(0, 1, 3, 2)).astype(bf16).reshape(
        B, KK, NN * PP
    )
    vq_b = vq.astype(bf16).reshape(B, L, MM * NN)

    scale_arr = np.asarray(scale, dtype=np.float32).reshape(1, 1)
    gamma_arr = np.asarray(ln_gamma, dtype=np.float32).reshape(1, DD)
    beta_arr = np.asarray(ln_beta, dtype=np.float32).reshape(1, DD)

    in_maps = [
        {
            "q": q_p[c],
            "k": k_p[c],
            "vq": vq_b[c],
            "vk": vk_b[c],
            "vexp": vexp_p[c],
            "scale": scale_arr,
            "ln_gamma": gamma_arr,
            "ln_beta": beta_arr,
        }
        for c in range(NCORES)
    ]
    trace = bool(os.environ.get("KERNEL_TRACE"))
    kw = {}
    if trace:
        kw = dict(trace=True, tmpdir=os.environ.get("KERNEL_TRACE_DIR") or None)
    res = bass_utils.run_bass_kernel_spmd(
        nc, in_maps, core_ids=list(range(NCORES)), **kw
    )
    if trace:
        global LAST_EXEC_NS, LAST_PROFILE_JSON
        LAST_EXEC_NS = res.exec_time_ns
        LAST_PROFILE_JSON = res.profile_json
    # un-permute output rows: [128, LT*MM] -> [L, MM]
    out_p = np.stack([res.results[c]["out"] for c in range(NCORES)], axis=0)
    out = (
        out_p.reshape(B, 128, LT, MM).transpose(0, 2, 1, 3).reshape(B, L, MM)
    )
    return np.ascontiguousarray(out.astype(np.float32))
